# revision 3
# baseline (speedup 1.0000x reference)
"""KMeansPalettizedConv2d on 8 TRN2 NeuronCores.

Strategy (data-parallel, per sharding hint): 4 images per core; the
256-entry lookup table, compressed int16 palette indices, and bias are
replicated to every core. This execution environment is instruction-
dispatch-bound (~40us per straight-line instruction, ~7us per For_i
replay, per-engine queues dispatch concurrently; FLOPs/bytes nearly
free), so the kernel minimizes dispatched instructions and spreads
issue across engine queues:

- Dequant on device: 16 GpSimd ap_gather instrs (4608-idx f32 table
  lookups) + 16 partition-strided SBUF->SBUF redistribution DMAs (also
  issued from the GpSimd queue) into lhsT layout, double-buffered so a
  following pass's dequant overlaps the current conv.
- Conv: one flat 28-iteration For_i over (image x 8-row tile) using a
  redundant 10-row-tile image layout that makes every offset affine in
  the loop var (rhs rows = v*10+ky, out cols = v*448). Body = 2
  cout-chunks x 18 accumulating f32 matmuls (self-loading weights; bf16
  would pay a separate Ldweights per matmul) into a single two-bank
  [128,2,512] PSUM tile, drained by ONE vector tensor_add against a
  host-built bias map and ONE 3D-AP output DMA issued from the
  otherwise-idle scalar queue. 1008 matmuls total, ~60 dispatched
  instructions in the loop.
- Host work is layout prep only: zero-padding, index re-layout
  (int32 -> wrapped int16), bias broadcast, output reassembly.
"""
import numpy as np

import concourse.mybir as mybir
import concourse.tile as tile
from concourse import bacc
from concourse.bass import ds
from concourse.bass_utils import run_bass_kernel_spmd

F32 = mybir.dt.float32
I16 = mybir.dt.int16

N_CORES = 8
N_IMG = 4
HP = 58
NI = 4608
_NC_CACHE = {}


def build_nc(loop_reps=1, flat=True):
    nc = bacc.Bacc("TRN2", target_bir_lowering=False, debug=False,
                   num_devices=N_CORES)
    xin = nc.dram_tensor("xin", [N_IMG, 2, 128, HP * HP], F32,
                         kind="ExternalInput")
    bmap = nc.dram_tensor("bmap", [128, 2, 448], F32, kind="ExternalInput")
    tab = nc.dram_tensor("tab", [128, 256], F32, kind="ExternalInput")
    widx = nc.dram_tensor("widx", [128, 16 * (NI // 16)], I16,
                          kind="ExternalInput")
    # out3[p, b, v*448 + k],  v = i*7 + rt
    out3 = nc.dram_tensor("out3", [128, 2, N_IMG * 3136], F32,
                          kind="ExternalOutput")

    with tile.TileContext(nc) as tc:
        with (
            tc.tile_pool(name="const", bufs=1) as constp,
            tc.tile_pool(name="imgp", bufs=2) as imgp,
            tc.tile_pool(name="wf", bufs=2) as wfp,
            tc.tile_pool(name="slab", bufs=1) as slabp,
            tc.tile_pool(name="ps", bufs=2, space="PSUM") as psp,
            tc.tile_pool(name="ob", bufs=2) as obp,
        ):
            bmap_sb = constp.tile([128, 2, 448], F32)
            nc.sync.dma_start(bmap_sb[:], bmap[:])
            tab_sb = constp.tile([128, 256], F32)
            nc.sync.dma_start(tab_sb[:], tab[:])
            idx_sb = constp.tile([128, 16 * (NI // 16)], I16)
            nc.sync.dma_start(idx_sb[:], widx[:])

            # redundant 10-row tiles: image i, row-tile rt at rows
            # [(i*7+rt)*10, +10) = input rows rt*8 .. rt*8+9
            ximg = []
            for a in range(2):
                t = imgp.tile([128, N_IMG * 7 * 10, HP], F32)
                for i in range(N_IMG):
                    for rt in range(7):
                        nc.sync.dma_start(
                            t[:, (i * 7 + rt) * 10:(i * 7 + rt) * 10 + 10, :],
                            xin[i, a][:, rt * 8 * HP:(rt * 8 + 10) * HP])
                ximg.append(t)

            def dequant(wf32):
                for r in range(16):
                    s = slabp.tile([128, NI], F32)
                    nc.gpsimd.ap_gather(
                        s[:], tab_sb[:],
                        idx_sb[:, r * (NI // 16):(r + 1) * (NI // 16)],
                        channels=128, num_elems=256, d=1, num_idxs=NI)
                    nc.gpsimd.dma_start(wf32[r::16, :], s[r::16, :])

            def conv(wf32):
                with tc.For_i(0, N_IMG * 7) as v:
                    ps = psp.tile([128, 2, 512], F32, tag="ps", name="ps")
                    for b in range(2):
                        for a in range(2):
                            for kk in range(9):
                                ky, kx = kk // 3, kk % 3
                                t_idx = b * 18 + a * 9 + kk
                                w_ap = wf32[:, t_idx * 128:(t_idx + 1) * 128]
                                rhs = ximg[a][:, ds(v * 10 + ky, 8),
                                              kx:kx + 56]
                                nc.tensor.matmul(
                                    ps[:, b, 0:448], w_ap, rhs,
                                    start=(a == 0 and kk == 0),
                                    stop=(a == 1 and kk == 8))
                    o2 = obp.tile([128, 2, 448], F32)
                    nc.vector.tensor_add(o2[:], ps[:, :, 0:448], bmap_sb[:])
                    nc.scalar.dma_start(out3[:, :, ds(v * 448, 448)], o2[:])

            for _ in range(loop_reps):
                wf32 = wfp.tile([128, 4608], F32)
                dequant(wf32)
                conv(wf32)
    nc.finalize()
    return nc


def prep_inputs(input, weight_idx, lookup_table, bias):
    input = np.asarray(input)
    weight_idx = np.asarray(weight_idx)
    lookup_table = np.asarray(lookup_table, dtype=np.float32)
    bias = np.asarray(bias, dtype=np.float32)

    xp = np.zeros((32, 256, HP, HP), np.float32)
    xp[:, :, 1:57, 1:57] = input
    xin = xp.reshape(32, 2, 128, HP * HP)

    bmap = np.ascontiguousarray(
        np.broadcast_to(bias.reshape(2, 128).T[:, :, None], (128, 2, 448))
        .astype(np.float32))

    A = weight_idx.reshape(2, 128, 2, 8, 16, 9)      # [b, co, a, g, r, kk]
    L = A.transpose(4, 3, 0, 2, 5, 1).reshape(16, 8, NI)  # [r, g, j]
    widx = (L.reshape(16, 8, NI // 16, 16)
             .transpose(0, 1, 3, 2)
             .reshape(16, 128, NI // 16)
             .transpose(1, 0, 2)
             .reshape(128, 16 * (NI // 16))
             .astype(np.int16))

    com = {
        "bmap": bmap,
        "widx": np.ascontiguousarray(widx),
        "tab": np.broadcast_to(lookup_table, (128, 256)).copy(),
    }
    return [{"xin": xin[c * N_IMG:(c + 1) * N_IMG], **com}
            for c in range(N_CORES)]


def run(in_maps, loop_reps=1, cores=None):
    if loop_reps not in _NC_CACHE:
        _NC_CACHE[loop_reps] = build_nc(loop_reps)
    if cores is None:
        cores = list(range(N_CORES))
    return run_bass_kernel_spmd(_NC_CACHE[loop_reps], in_maps[:len(cores)],
                                core_ids=cores)


def kernel(input, weight_idx, lookup_table, bias):
    in_maps = prep_inputs(input, weight_idx, lookup_table, bias)
    res = run(in_maps)
    outs = [res.results[c]["out3"] for c in range(N_CORES)]
    full = np.stack(outs, axis=0)                # [8, 128, 2, 4*3136]
    full = full.reshape(N_CORES, 128, 2, N_IMG, 3136)
    full = full.transpose(0, 3, 2, 1, 4)         # [8, 4, 2, 128, 3136]
    return np.ascontiguousarray(full).reshape(32, 256, 56, 56)
